# revision 1
# baseline (speedup 1.0000x reference)
"""Trainium2 Bass kernel: banded additive attention (window 64).

reference semantics (B=4, T=1024, D=512, U=32, WIDTH=64):
  q = x @ Wt ; k = x @ Wx
  e[b,t,j] = exp(Wa . tanh(q[b,t]+k[b,j]+bh) + ba) for j in [t-32, t+31]
  a = e / (sum_j e + 1e-7) ; v = a @ x

Sharding: 8 NeuronCores = (batch b, T-half). Each core computes 512 query
rows with a 32-row halo; weights replicated. Inputs are packed/cast on the
host into SBUF-shaped blocks so each needs a single DMA.

Per-core pipeline (Tile-scheduled):
  1. PE projections qT/kT (contract D in 4 chunks vs host-pretransposed xT).
  2. Single-DMA replicate into diagonal layout q4[128,2048] / k4[128,576]
     (partition p = 32g+u, diagonal d' = 4r+g; k group g pre-shifted by g).
  3. 4x (DVE add [128,2048] -> ACT tanh -> 4 PE matmuls with a sliced
     "wide Wa" lhsT accumulating all 64 diagonals into E[64,512] PSUM).
  4. ACT exp(+ba) -> mask multiply writing B0 in permuted (sigmaA) order.
  5. Diagonal->banded shear: 2-stage radix-8 butterfly of PE shift-matmuls
     (lhsT = column slices of a wide 0/1 diagonal-band constant); the
     column permutations ride on strided DVE copies.
  6. Value: per 64-row block, B[:,blk].T @ xe-block where xe carries a
     ones-column so the softmax denominator falls out of the same matmul.
  7. DVE reciprocal + per-partition scale, DMA out.
"""
import os
import sys

sys.path.insert(0, "/opt/trn_rl_repo")

import numpy as np
import ml_dtypes  # noqa: E402
import concourse.bass as bass  # noqa: E402
import concourse.mybir as mybir  # noqa: E402
from concourse import bacc, tile  # noqa: E402
from concourse.ap import AP  # noqa: E402
from concourse.bass_utils import run_bass_kernel_spmd  # noqa: E402

F32 = mybir.dt.float32
BF16 = mybir.dt.bfloat16
ActFn = mybir.ActivationFunctionType

B, T, D, U = 4, 1024, 512, 32
WIDTH = 64
EPS = 1e-7
T_LOC = 512
HALO = 576
NBLK = 8
NCORES = 8

_CDT = BF16 if os.environ.get("ATTN_CDT", "bf16") == "bf16" else F32


def _np_dt(cdt):
    return ml_dtypes.bfloat16 if cdt == BF16 else np.float32


def _emit(nc, tc, cdt, xt, xe, wws, mbb, vout):
    from contextlib import ExitStack
    ctx = ExitStack()
    with ctx:
        cpool = ctx.enter_context(tc.tile_pool(name="consts", bufs=1))
        work = ctx.enter_context(tc.tile_pool(name="work", bufs=1))
        tpool = ctx.enter_context(tc.tile_pool(name="tanh", bufs=2))
        opool = ctx.enter_context(tc.tile_pool(name="outs", bufs=4))

        # ---------- Phase 0: ACT table preload + single-DMA loads ----------
        dummy = cpool.tile([1, 1], F32, tag="dummy")
        nc.vector.memset(dummy[:], 0.0)
        nc.scalar.activation(dummy[:], dummy[:], ActFn.Exp)

        wws_sb = cpool.tile([128, 1147], cdt, tag="wws_sb")
        nc.sync.dma_start(wws_sb[:, 0:640], wws[:, 0:640])
        nc.sync.dma_start(wws_sb[:, 640:1147], wws[:, 640:1147])
        xt01 = cpool.tile([128, 2 * HALO], cdt, tag="xt01")
        nc.gpsimd.dma_start(xt01[:], xt[:, 0:2 * HALO])
        xt23 = cpool.tile([128, 2 * HALO], cdt, tag="xt23")
        nc.gpsimd.dma_start(xt23[:], xt[:, 2 * HALO:])

        def xt_chunk(c, lo, hi):
            t = xt01 if c < 2 else xt23
            base = HALO * (c % 2)
            return t[:, base + lo:base + hi]
        mbb_sb = cpool.tile([128, 2], F32, tag="mbb_sb")
        nc.sync.dma_start(mbb_sb[:], mbb[:])
        xe_all = cpool.tile([128, 8 * 513], cdt, tag="xe_all")
        nc.gpsimd.dma_start(xe_all[:], xe[:])
        w_all = wws_sb[:, 0:640]
        wa_sb = wws_sb[:, 640:764]
        # sh: [128, 255] 0/1 band, sh[k, c] = (c == k + 127)
        sh_sb = wws_sb[:, 764:1019]
        # edge-mask rank-32 factors: R_lo/R_hi [32, 32] in cdt
        rlo_sb = wws_sb[0:32, 1019:1051]
        rhi_sb = wws_sb[0:32, 1051:1083]
        ba_sb = mbb_sb[0:64, 0:1]
        bh4_sb = mbb_sb[:, 1:2]

        # ---------- Phase 1: projections ----------
        with tc.tile_pool(name="pp1", bufs=1, space="PSUM") as pp1:
            kTa_ps = pp1.tile([U, 512], F32, tag="kTa")
            kTb_ps = pp1.tile([U, 64], F32, tag="kTb")
            for c in range(4):
                nc.tensor.matmul(kTa_ps[:],
                                 w_all[:, 512 + 32 * c:512 + 32 * c + 32],
                                 xt_chunk(c, 0, 512),
                                 start=(c == 0), stop=(c == 3))
            for c in range(4):
                nc.tensor.matmul(kTb_ps[:],
                                 w_all[:, 512 + 32 * c:512 + 32 * c + 32],
                                 xt_chunk(c, 512, 576),
                                 start=(c == 0), stop=(c == 3))
            # ---- q4s directly: lhsT = W4t chunks (Wt replicated x4 in M) ----
            q4_ps = pp1.tile([128, T_LOC], F32, tag="q4_ps")
            for c in range(4):
                nc.tensor.matmul(q4_ps[:], w_all[:, 128 * c:128 * c + 128],
                                 xt_chunk(c, 32, 32 + T_LOC),
                                 start=(c == 0), stop=(c == 3))
            q4s = work.tile([128, T_LOC], cdt, tag="q4s")
            nc.vector.tensor_scalar_add(q4s[:], q4_ps[:], bh4_sb)
            # k4[32g+u, c] = kT[u, c+g]: stage kT to SBUF, then 4 shifted-rep
            # matmuls (lhsT = sh slices) accumulate k4 in PSUM, copy out wide
            kTb_sb = work.tile([U, HALO], cdt, tag="kTb_sb")
            nc.vector.tensor_copy(kTb_sb[:, 0:512], kTa_ps[:])
            nc.scalar.copy(kTb_sb[:, 512:576], kTb_ps[:])
            k4_ps = pp1.tile([128, 1024], F32, tag="k4_ps")
            for g in range(4):
                lhsT = sh_sb[0:32, 127 - 32 * g:255 - 32 * g]
                nc.tensor.matmul(k4_ps[:, 0:512],
                                 lhsT, kTb_sb[:, g:g + 512],
                                 start=(g == 0), stop=(g == 3))
            for g in range(4):
                lhsT = sh_sb[0:32, 127 - 32 * g:255 - 32 * g]
                nc.tensor.matmul(k4_ps[:, 512:573],
                                 lhsT, kTb_sb[:, 512 + g:573 + g],
                                 start=(g == 0), stop=(g == 3))
            k4 = work.tile([128, HALO], cdt, tag="k4")
            nc.vector.tensor_copy(k4[:, 0:573], k4_ps[:, 0:573])

        with tc.tile_pool(name="pp2", bufs=2, space="PSUM") as pp2:
            E_ps = pp2.tile([64, T_LOC], F32, tag="stage")

            # ---------- Phase 3: scores (uneven batches 2,4,4,4,2:
            # small first batch starts tanh earlier, small last batch
            # finishes the E-gating matmuls earlier) ----------
            for (s0, nsl) in [(0,1),(1,3),(4,4),(8,4),(12,3),(15,1)]:
                tin = tpool.tile([128, 512 * nsl], cdt, tag="tin")
                k4ap = AP(k4[:].tensor, 4 * s0,
                          [[HALO, 128], [4, nsl], [1, T_LOC]])
                q4ap = AP(q4s[:].tensor, 0,
                          [[T_LOC, 128], [0, nsl], [1, T_LOC]])
                nc.vector.tensor_add(
                    tin[:].rearrange("p (a t) -> p a t", a=nsl),
                    q4ap, k4ap)
                tout = tpool.tile([128, 512 * nsl], cdt, tag="tout")
                nc.scalar.activation(tout[:], tin[:], ActFn.Tanh)
                for j in range(nsl):
                    r = s0 + j
                    nc.tensor.matmul(E_ps[:],
                                     wa_sb[:, 60 - 4 * r:124 - 4 * r],
                                     tout[:, 512 * j:512 * j + 512],
                                     start=(r == 0), stop=False)

            # edge mask as two rank-32 accumulations: E += -30 on invalid j
            nc.tensor.matmul(E_ps[:, 0:32], sh_sb[0:32, 127:191],
                             rlo_sb, start=False, stop=False)
            nc.tensor.matmul(E_ps[:, 480:512], sh_sb[0:32, 95:159],
                             rhi_sb, start=False, stop=True)

            # ---------- Phase 4: exp straight to B0 in sigmaA order -------
            # t = 64m + 8a + b ; sigmaA col = 64b+8m+a ; sigmaB col = 64a+8m+b
            B0 = work.tile([128, T_LOC], cdt, tag="B0")
            nc.vector.memset(B0[64:128, :], 0.0)
            b0_out = AP(B0[:].tensor, 0, [[T_LOC, 64], [64, 8], [8, 8], [1, 8]])
            e_in = AP(E_ps[:].tensor, 0, [[T_LOC, 64], [1, 8], [64, 8], [8, 8]])
            nc.scalar.activation(b0_out, e_in, ActFn.Exp, bias=ba_sb)

            # ---------- Phase 5: shear butterfly ----------
            P1 = pp2.tile([128, T_LOC], F32, tag="stage")
            S1 = work.tile([128, T_LOC], cdt, tag="S1")
            P2 = pp2.tile([128, T_LOC], F32, tag="stage")
            Bsb = work.tile([128, T_LOC], cdt, tag="Bsb")
            for b in range(8):
                nc.tensor.matmul(P1[:, 64 * b:64 * b + 64],
                                 sh_sb[:, 127 - b:255 - b],
                                 B0[:, 64 * b:64 * b + 64],
                                 start=True, stop=True)
            s1_out = AP(S1[:].tensor, 0,
                        [[T_LOC, 128], [1, 8], [8, 8], [64, 8]])
            p1_in = AP(P1[:].tensor, 0,
                       [[T_LOC, 128], [64, 8], [8, 8], [1, 8]])
            nc.scalar.copy(s1_out, p1_in)
            for a in range(8):
                nc.tensor.matmul(P2[:, 64 * a:64 * a + 64],
                                 sh_sb[:, 127 - 8 * a:255 - 8 * a],
                                 S1[:, 64 * a:64 * a + 64],
                                 start=True, stop=True)
            bsb_out = AP(Bsb[:].tensor, 0,
                         [[T_LOC, 128], [8, 8], [64, 8], [1, 8]])
            p2_in = AP(P2[:].tensor, 0,
                       [[T_LOC, 128], [64, 8], [8, 8], [1, 8]])
            nc.scalar.copy(bsb_out, p2_in)

            # ---------- Phase 6: value + normalize ----------
            vstack = ExitStack()
            vpool = vstack.enter_context(
                tc.tile_pool(name="vpsum", bufs=3, space="PSUM"))
            for mp in range(4):
                vp = vpool.tile([128, 1024], F32, tag="vp")
                for h in range(2):
                    m = 2 * mp + h
                    lhsT = Bsb[:, 64 * m:64 * m + 64]
                    rhs = xe_all[:, 513 * m:513 * m + 513]
                    nc.tensor.matmul(vp[64 * h:64 * h + 64, 0:512],
                                     lhsT, rhs[:, 0:512],
                                     start=True, stop=True)
                    nc.tensor.matmul(vp[64 * h:64 * h + 64, 512:513],
                                     lhsT, rhs[:, 512:513],
                                     start=True, stop=True)
                # EPS dropped: s >= 64*exp(-~5) makes 1e-7 negligible
                rcol = opool.tile([128, 1], F32, tag="rcol")
                nc.vector.reciprocal(rcol[:], vp[:, 512:513])
                vsb = opool.tile([128, D], vout.dtype, tag="vsb")
                if mp % 2 == 0:
                    nc.scalar.activation(vsb[:], vp[:, 0:512], ActFn.Copy,
                                         scale=rcol[:])
                else:
                    nc.vector.tensor_scalar_mul(vsb[:], vp[:, 0:512],
                                                rcol[:])
                nc.sync.dma_start(vout[128 * mp:128 * mp + 128, :], vsb[:])
            vstack.close()


def build_nc(cdt=_CDT):
    nc = bacc.Bacc("TRN2", target_bir_lowering=False)
    xt = nc.dram_tensor("xt", [128, 4 * HALO], cdt, kind="ExternalInput")
    xe = nc.dram_tensor("xe", [128, 8 * 513], cdt, kind="ExternalInput")
    wws = nc.dram_tensor("wws", [128, 1147], cdt, kind="ExternalInput")
    mbb = nc.dram_tensor("mbb", [128, 2], F32, kind="ExternalInput")
    vout = nc.dram_tensor("v", [T_LOC, D], _CDT if os.environ.get("ATTN_VOUT", "bf16") == "bf16" else F32, kind="ExternalOutput")
    with tile.TileContext(nc) as tc:
        _emit(nc, tc, cdt, xt, xe, wws, mbb, vout)
    nc.compile()
    return nc


# ---------------- host-side prep ----------------

def prep_core_inputs(x, Wt, Wx, bh, Wa, ba, core, cdt=_CDT):
    ndt = _np_dt(cdt)
    b, half = core // 2, core % 2
    t0 = half * T_LOC
    lo, hi = t0 - 32, t0 + 544
    pad_lo, pad_hi = max(0, -lo), max(0, hi - T)
    xs = x[b, max(0, lo):min(T, hi), :]
    x_halo = np.pad(xs, ((pad_lo, pad_hi), (0, 0)))     # [576, 512]

    # xt: [128, 4*576], chunk c = x_halo[:, 128c:128c+128].T
    xt = np.empty((128, 4 * HALO), np.float32)
    for c in range(4):
        xt[:, HALO * c:HALO * (c + 1)] = x_halo[:, 128 * c:128 * c + 128].T
    # xe: [128, 8*513], block m = rows [64m, 64m+128) with ones column
    xe_rows = np.concatenate(
        [x_halo, np.ones((HALO, 1), np.float32)], 1)    # [576, 513]
    xe = np.empty((128, 8 * 513), np.float32)
    for m in range(NBLK):
        xe[:, 513 * m:513 * (m + 1)] = xe_rows[64 * m:64 * m + 128, :]
    # wws: [128, 1147] = w[640] | wa_wide[124] | sh[255] | Rlo[32] | Rhi[32]
    #                    | spare[64]
    wws = np.zeros((128, 1147), np.float32)
    for c in range(4):
        wws[:, 128 * c:128 * c + 128] = np.tile(Wt[128 * c:128 * c + 128, :],
                                                (1, 4))
        wws[:, 512 + 32 * c:512 + 32 * c + 32] = Wx[128 * c:128 * c + 128, :]
    for g in range(4):
        wws[32 * g:32 * g + 32, 640 + 60 + g] = Wa[:, 0]
    kk = np.arange(128)
    wws[kk, 764 + kk + 127] = 1.0
    # edge-mask factors: E[d', t] += -30 where j = t0 + t + d' - 32 invalid.
    # left edge (t0 == 0):  invalid iff t + d' < 32  (d' = k in [0,32))
    # right edge (t0+512 == T): invalid iff t + d' > 543 (d' = k+32)
    ks = np.arange(32)[:, None]
    ts = np.arange(32)[None, :]
    if t0 == 0:
        wws[0:32, 1019:1051] = np.where(ts < 32 - ks, -30.0, 0.0)
    if t0 + T_LOC == T:
        wws[0:32, 1051:1083] = np.where((480 + ts) + (ks + 32) > 543,
                                        -30.0, 0.0)
    # mbb: [128, 2] = ba (rows 0-63) | bh4
    mbb = np.zeros((128, 2), np.float32)
    mbb[0:64, 0] = float(np.asarray(ba).reshape(-1)[0])
    mbb[:, 1] = np.tile(np.asarray(bh, np.float32), 4)

    return {
        "xt": xt.astype(ndt),
        "xe": xe.astype(ndt),
        "wws": wws.astype(ndt),
        "mbb": mbb,
    }


_NC_CACHE = {}


def _get_nc(cdt=_CDT):
    key = str(cdt)
    if key not in _NC_CACHE:
        _NC_CACHE[key] = build_nc(cdt)
    return _NC_CACHE[key]


def kernel(x, Wt, Wx, bh, Wa, ba, _trace=False):
    x = np.asarray(x, np.float32)
    Wt = np.asarray(Wt, np.float32)
    Wx = np.asarray(Wx, np.float32)
    bh = np.asarray(bh, np.float32)
    Wa = np.asarray(Wa, np.float32)
    ba = np.asarray(ba, np.float32)
    nc = _get_nc()
    in_maps = [prep_core_inputs(x, Wt, Wx, bh, Wa, ba, c)
               for c in range(NCORES)]
    res = run_bass_kernel_spmd(nc, in_maps, core_ids=list(range(NCORES)),
                               trace=_trace)
    out = np.empty((B, T, D), np.float32)
    for c in range(NCORES):
        b, half = c // 2, c % 2
        out[b, half * T_LOC:(half + 1) * T_LOC, :] = np.asarray(
            res.results[c]["v"], np.float32)
    if _trace:
        return out, res
    return out



# revision 17
# speedup vs baseline: 1.0528x; 1.0528x over previous
"""Trainium2 Bass kernel: banded additive attention (window 64).

reference semantics (B=4, T=1024, D=512, U=32, WIDTH=64):
  q = x @ Wt ; k = x @ Wx
  e[b,t,j] = exp(Wa . tanh(q[b,t]+k[b,j]+bh) + ba) for j in [t-32, t+31]
  a = e / (sum_j e + 1e-7) ; v = a @ x

Sharding: 8 NeuronCores = (batch b, T-half). Each core computes 512 query
rows with a 32-row halo; weights replicated. Inputs are packed/cast on the
host into SBUF-shaped blocks so each needs a single DMA.

Per-core pipeline (Tile-scheduled), hand-interleaved in two 256-query
halves so the exp/shear/value/output tail of half 0 hides under the
scoring of half 1:
  0. PE warmup chain keeps the tensor engine p-state ramped through the
     input-DMA wait; x and the q/k weights ship as fp8e4m3 to halve the
     critical input DMA.
  1. PE projections: kT split [0:322]/[322:576] (contract D in 4 chunks vs
     host-pretransposed xT), q4 in column halves; shifted-replicate
     matmuls (lhsT = slices of a 0/1 band constant) build the diagonal
     layout k4[32g+u, c] = kT[u, c+g]; PSUM->SBUF copies split so half-0
     scoring starts as early as possible.
  2. Per half: 4x (DVE add -> ACT tanh -> PE matmuls with sliced "wide Wa"
     lhsT accumulating 64 diagonals into E[64,256] PSUM).
  3. ACT exp(+ba) -> B0 in sigmaA order; radix-8 shear butterfly (shift
     matmuls + column-permute copies) -> banded B.
  4. Value: per 64-row block, B-block.T @ xe-block with a ones-column so
     the softmax denominator falls out of the same matmul; per-partition
     reciprocal scale; per-128-row output DMA.
"""
import os
import sys

sys.path.insert(0, "/opt/trn_rl_repo")

import numpy as np
import ml_dtypes  # noqa: E402
import concourse.bass as bass  # noqa: E402
import concourse.mybir as mybir  # noqa: E402
from concourse import bacc, tile  # noqa: E402
from concourse.ap import AP  # noqa: E402
from concourse.bass_utils import run_bass_kernel_spmd  # noqa: E402

F32 = mybir.dt.float32
BF16 = mybir.dt.bfloat16
FP8 = mybir.dt.float8e4
ActFn = mybir.ActivationFunctionType

B, T, D, U = 4, 1024, 512, 32
WIDTH = 64
EPS = 1e-7
T_LOC = 512
HALF = 256
HALO = 576
NBLK = 8
NCORES = 8

_CDT = BF16 if os.environ.get("ATTN_CDT", "bf16") == "bf16" else F32
_XDT = FP8 if os.environ.get("ATTN_XDT", "fp8") == "fp8" else _CDT
_NWARM = int(os.environ.get("ATTN_NWARM", "11"))

# kT column split points: k4[:,0:280) needs kT[:,0:283); score batches
# b1+b2 of half 0 read k4 cols [0:280) only
KTS1 = 283
KTS = 322
K4S1 = 280
K4S = 319

# wwr layout: wa_wide[124] | sh[255] | rlo[32] | rhi[32]
WA0, SH0, RLO0, RHI0, WWR = 0, 124, 379, 411, 443

# score batches per half: (slice0, nslices); h1 descends so the last
# E-matmul group (which gates exp) is small
BATCHES = [(0, 2), (2, 4), (6, 5), (11, 5)]
BATCHES1 = [(0, 5), (5, 5), (10, 4), (14, 2)]


def _np_dt(dt_):
    if dt_ == FP8:
        return ml_dtypes.float8_e4m3
    return ml_dtypes.bfloat16 if dt_ == BF16 else np.float32


def _emit(nc, tc, cdt, xt, xe, wwq, wwr, mbb, vout):
    from contextlib import ExitStack
    ctx = ExitStack()
    with ctx:
        cpool = ctx.enter_context(tc.tile_pool(name="consts", bufs=1))
        work = ctx.enter_context(tc.tile_pool(name="work", bufs=1))
        tpool = ctx.enter_context(tc.tile_pool(name="tanh", bufs=2))
        spool = ctx.enter_context(tc.tile_pool(name="shear", bufs=2))
        opool = ctx.enter_context(tc.tile_pool(name="outs", bufs=4))
        xdt = xt.dtype

        # ---------- Phase 0: ACT table preload + DMA loads + PE warmup -----
        dummy = cpool.tile([1, 1], F32, tag="dummy")
        nc.vector.memset(dummy[:], 0.0)
        nc.scalar.activation(dummy[:], dummy[:], ActFn.Exp)

        wwq_sb = cpool.tile([128, 640], cdt, tag="wwq_sb")
        nc.sync.dma_start(wwq_sb[:], wwq[:])
        xt_sb = cpool.tile([128, 4 * HALO], xdt, tag="xt_sb")
        nc.gpsimd.dma_start(xt_sb[:, 0:2 * HALO], xt[:, 0:2 * HALO])
        nc.sync.dma_start(xt_sb[:, 2 * HALO:], xt[:, 2 * HALO:])
        mbb_sb = cpool.tile([128, 2], F32, tag="mbb_sb")
        nc.sync.dma_start(mbb_sb[:], mbb[:])
        wwr_sb = cpool.tile([128, WWR], cdt, tag="wwr_sb")
        nc.sync.dma_start(wwr_sb[:], wwr[:])
        xe_all = cpool.tile([128, 8 * 513], cdt, tag="xe_all")
        nc.gpsimd.dma_start(xe_all[:], xe[:])

        def xt_chunk(c, lo, hi):
            return xt_sb[:, HALO * c + lo:HALO * c + hi]

        wa_sb = wwr_sb[:, WA0:WA0 + 124]
        sh_sb = wwr_sb[:, SH0:SH0 + 255]
        rlo_sb = wwr_sb[0:32, RLO0:RLO0 + 32]
        rhi_sb = wwr_sb[0:32, RHI0:RHI0 + 32]
        ba_sb = mbb_sb[0:64, 0:1]
        bh4_sb = mbb_sb[:, 1:2]

        # PE warmup: keep the tensor engine continuously busy through the
        # input-DMA wait so the p-state ramp is done before real matmuls.
        warm = cpool.tile([128, 256], cdt, tag="warm")
        nc.vector.memset(warm[:], 0.0)
        wstack = ExitStack()
        wps = wstack.enter_context(tc.tile_pool(name="wps", bufs=1,
                                                space="PSUM"))
        wp = wps.tile([128, 256], F32, tag="wp")
        for _ in range(_NWARM):
            nc.tensor.matmul(wp[:], warm[:, 0:128], warm[:],
                             start=True, stop=True)
        wstack.close()

        # ---------- Phase 1: projections (critical chain to first tanh) ---
        estack = ExitStack()
        ppe = estack.enter_context(tc.tile_pool(name="ppe", bufs=2,
                                                space="PSUM"))
        pstack = ExitStack()
        pp1 = pstack.enter_context(tc.tile_pool(name="pp1", bufs=1,
                                                space="PSUM"))
        kTa1_ps = pp1.tile([U, KTS1], F32, tag="kTa1")       # kT[:, 0:283]
        kTa2_ps = pp1.tile([U, KTS - KTS1], F32, tag="kTa2")  # [283:322]
        kTb_ps = pp1.tile([U, HALO - KTS], F32, tag="kTb")   # kT[:, 322:576]
        q4_ps = pp1.tile([128, T_LOC], F32, tag="q4_ps")
        k4_ps = pp1.tile([128, 1024], F32, tag="k4_ps")
        kT_sb = work.tile([U, HALO], cdt, tag="kT_sb")
        q4s = work.tile([128, T_LOC], cdt, tag="q4s")
        k4 = work.tile([128, HALO], cdt, tag="k4")

        def wx(c):
            return wwq_sb[:, 512 + 32 * c:512 + 32 * c + 32]

        # chunk-pair interleave: c0/c1 arrive first (Pool queue), c2/c3 on
        # the SP queue land ~400ns later
        for c in range(2):
            nc.tensor.matmul(kTa1_ps[:], wx(c), xt_chunk(c, 0, KTS1),
                             start=(c == 0), stop=False)
        for c in range(2):
            nc.tensor.matmul(q4_ps[:, 0:HALF],
                             wwq_sb[:, 128 * c:128 * c + 128],
                             xt_chunk(c, 32, 32 + HALF),
                             start=(c == 0), stop=False)
        for c in range(2, 4):
            nc.tensor.matmul(kTa1_ps[:], wx(c), xt_chunk(c, 0, KTS1),
                             start=False, stop=(c == 3))
        for c in range(2, 4):
            nc.tensor.matmul(q4_ps[:, 0:HALF],
                             wwq_sb[:, 128 * c:128 * c + 128],
                             xt_chunk(c, 32, 32 + HALF),
                             start=False, stop=(c == 3))
        for c in range(4):
            nc.tensor.matmul(kTa2_ps[:], wx(c), xt_chunk(c, KTS1, KTS),
                             start=(c == 0), stop=(c == 3))
        nc.vector.tensor_copy(kT_sb[:, 0:KTS1], kTa1_ps[:])    # DVE
        # k4a-c1: k4[32g+u, c] = kT[u, c+g] for c in [0:280)
        for g in range(4):
            lhsT = sh_sb[0:32, 127 - 32 * g:255 - 32 * g]
            nc.tensor.matmul(k4_ps[:, 0:K4S1],
                             lhsT, kT_sb[:, g:g + K4S1],
                             start=(g == 0), stop=(g == 3))
        for c in range(4):  # PE filler while DVE copies
            nc.tensor.matmul(kTb_ps[:], wx(c), xt_chunk(c, KTS, HALO),
                             start=(c == 0), stop=(c == 3))
        nc.vector.tensor_scalar_add(q4s[:, 0:HALF], q4_ps[:, 0:HALF],
                                    bh4_sb)                    # DVE
        nc.scalar.copy(kT_sb[:, KTS:HALO], kTb_ps[:])          # ACT (idle)
        nc.vector.tensor_copy(k4[:, 0:K4S1], k4_ps[:, 0:K4S1])  # DVE
        for c in range(4):
            nc.tensor.matmul(q4_ps[:, HALF:T_LOC],
                             wwq_sb[:, 128 * c:128 * c + 128],
                             xt_chunk(c, 32 + HALF, 32 + T_LOC),
                             start=(c == 0), stop=(c == 3))

        # ---------- Phases 2-4, hand-interleaved over halves ----------
        E = [None, None]
        B0 = [None, None]
        S1 = [None, None]
        P1 = [None, None]
        P2 = [None, None]
        Bsb = [None, None]
        vp = {}

        def score_batch(h, s0, nsl):
            if E[h] is None:
                E[h] = ppe.tile([64, HALF], F32, tag="E", name="E")
            tin = tpool.tile([128, HALF * nsl], cdt, tag="tin")
            k4ap = AP(k4[:].tensor, 4 * s0 + HALF * h,
                      [[HALO, 128], [4, nsl], [1, HALF]])
            q4ap = AP(q4s[:].tensor, HALF * h,
                      [[T_LOC, 128], [0, nsl], [1, HALF]])
            nc.vector.tensor_add(
                tin[:].rearrange("p (a t) -> p a t", a=nsl), q4ap, k4ap)
            tout = tpool.tile([128, HALF * nsl], cdt, tag="tout")
            nc.scalar.activation(tout[:], tin[:], ActFn.Tanh)
            for j in range(nsl):
                r = s0 + j
                nc.tensor.matmul(E[h][:],
                                 wa_sb[:, 60 - 4 * r:124 - 4 * r],
                                 tout[:, HALF * j:HALF * j + HALF],
                                 start=(r == 0), stop=False)

        def edge(h):
            # E += -30 on invalid j (zero matrices on interior cores)
            if h == 0:
                nc.tensor.matmul(E[0][:, 0:32], sh_sb[0:32, 127:191],
                                 rlo_sb, start=False, stop=True)
            else:
                nc.tensor.matmul(E[1][:, 224:256], sh_sb[0:32, 95:159],
                                 rhi_sb, start=False, stop=True)

        def exp_phase(h):
            B0[h] = spool.tile([64, HALF], cdt, tag="B0", name="B0")
            nc.scalar.activation(B0[h][:], E[h][:], ActFn.Exp, bias=ba_sb)

        def shear_mm(h):
            # one-stage shear: column t = 64m+8a+b needs shift s = 8a+b.
            # 64 tiny matmuls, one per s; rhs = B0 cols {64m + s} (stride 64)
            P1[h] = pps.tile([128, HALF], F32, tag="P1", name="P1")
            if os.environ.get("ATTN_SHEAR1", "1") == "1":
                for s in range(64):
                    rhs = AP(B0[h][:].tensor, s, [[HALF, 64], [64, 4]])
                    nc.tensor.matmul(P1[h][:, 4 * s:4 * s + 4],
                                     sh_sb[0:64, 127 - s:255 - s], rhs,
                                     start=True, stop=True)
            else:
                for s in range(64):
                    for i in range(4):
                        nc.tensor.matmul(
                            P1[h][:, 4 * s + i:4 * s + i + 1],
                            sh_sb[0:64, 127 - s:255 - s],
                            B0[h][:, 64 * i + s:64 * i + s + 1],
                            start=True, stop=True)

        def shear_copy(h, half2):
            # Bsb col 64m+8a+b <- P1 col 4(8a+b)+m, iter (a,b,m); emitted in
            # two m-pair chunks so the first value group starts earlier
            if half2 == 0:
                Bsb[h] = spool.tile([128, HALF], cdt, tag="Bsb", name="Bsb")
            bsb_out = AP(Bsb[h][:].tensor, 128 * half2,
                         [[HALF, 128], [8, 8], [1, 8], [64, 2]])
            p_in = AP(P1[h][:].tensor, 2 * half2,
                      [[HALF, 128], [32, 8], [4, 8], [1, 2]])
            nc.vector.tensor_copy(bsb_out, p_in)

        def value_mm(h, mp):
            t = vpool.tile([128, 1024], F32, tag="vp", name="vp")
            vp[(h, mp)] = t
            for h2 in range(2):  # denominators first so recip is off-path
                m2 = 2 * mp + h2
                m = 4 * h + m2
                nc.tensor.matmul(t[64 * h2:64 * h2 + 64, 512:513],
                                 Bsb[h][:, 64 * m2:64 * m2 + 64],
                                 xe_all[:, 513 * m + 512:513 * m + 513],
                                 start=True, stop=True)
            value_recip(h, mp)
            for h2 in range(2):
                m2 = 2 * mp + h2
                m = 4 * h + m2
                nc.tensor.matmul(t[64 * h2:64 * h2 + 64, 0:512],
                                 Bsb[h][:, 64 * m2:64 * m2 + 64],
                                 xe_all[:, 513 * m:513 * m + 512],
                                 start=True, stop=True)

        rcols = {}

        def value_recip(h, mp):
            t = vp[(h, mp)]
            # EPS dropped: s >= 64*exp(-~5) makes 1e-7 negligible
            rcol = opool.tile([128, 1], F32, tag="rcol", name="rcol")
            nc.vector.reciprocal(rcol[:], t[:, 512:513])
            rcols[(h, mp)] = rcol

        def value_scale(h, mp, scale_eng, vsb, col):
            t = vp[(h, mp)]
            rcol = rcols[(h, mp)]
            dst = vsb[:, col:col + D]
            if scale_eng == "act":
                nc.scalar.activation(dst, t[:, 0:512], ActFn.Copy,
                                     scale=rcol[:])
            else:
                nc.vector.tensor_scalar_mul(dst, t[:, 0:512], rcol[:])

        def value_out(h, mp, scale_eng):
            vsb = opool.tile([128, D], vout.dtype, tag="vsb", name="vsb")
            value_scale(h, mp, scale_eng, vsb, 0)
            row = 256 * h + 128 * mp
            nc.sync.dma_start(vout[row:row + 128, :], vsb[:])

        # --- interleaved emission ---
        score_batch(0, *BATCHES[0])
        score_batch(0, *BATCHES[1])
        nc.vector.tensor_copy(kT_sb[:, KTS1:KTS], kTa2_ps[:])
        for g in range(4):  # k4a-c2 shifts (need kTa2 copy above)
            lhsT = sh_sb[0:32, 127 - 32 * g:255 - 32 * g]
            nc.tensor.matmul(k4_ps[:, K4S1:K4S],
                             lhsT, kT_sb[:, K4S1 + g:K4S + g],
                             start=(g == 0), stop=(g == 3))
        nc.vector.tensor_copy(k4[:, K4S1:K4S], k4_ps[:, K4S1:K4S])
        score_batch(0, *BATCHES[2])
        for g in range(4):  # k4b shifts (need kTa2 + kTb copies);
            lhsT = sh_sb[0:32, 127 - 32 * g:255 - 32 * g]
            nc.tensor.matmul(k4_ps[:, K4S:512],  # psum-bank split at 512
                             lhsT, kT_sb[:, K4S + g:512 + g],
                             start=(g == 0), stop=(g == 3))
        for g in range(4):
            lhsT = sh_sb[0:32, 127 - 32 * g:255 - 32 * g]
            nc.tensor.matmul(k4_ps[:, 512:573],
                             lhsT, kT_sb[:, 512 + g:573 + g],
                             start=(g == 0), stop=(g == 3))
        score_batch(0, *BATCHES[3])
        nc.vector.tensor_copy(k4[:, K4S:573], k4_ps[:, K4S:573])
        nc.vector.tensor_scalar_add(q4s[:, HALF:T_LOC],
                                    q4_ps[:, HALF:T_LOC], bh4_sb)
        edge(0)
        pstack.close()
        pps = estack.enter_context(tc.tile_pool(name="pps", bufs=1,
                                                space="PSUM"))
        vpool = estack.enter_context(tc.tile_pool(name="vpsum", bufs=2,
                                                  space="PSUM"))

        score_batch(1, *BATCHES1[0])
        exp_phase(0)
        shear_mm(0)
        score_batch(1, *BATCHES1[1])
        shear_copy(0, 0)
        shear_copy(0, 1)
        score_batch(1, *BATCHES1[2])
        score_batch(1, *BATCHES1[3])
        edge(1)
        value_mm(0, 0)
        value_out(0, 0, "dve")
        value_mm(0, 1)
        value_out(0, 1, "act")
        exp_phase(1)
        shear_mm(1)
        shear_copy(1, 0)
        value_mm(1, 0)
        shear_copy(1, 1)
        value_mm(1, 1)
        value_out(1, 0, "dve")
        value_out(1, 1, "act")
        estack.close()


def build_nc(cdt=_CDT):
    nc = bacc.Bacc("TRN2", target_bir_lowering=False)
    xt = nc.dram_tensor("xt", [128, 4 * HALO], _XDT, kind="ExternalInput")
    xe = nc.dram_tensor("xe", [128, 8 * 513], cdt, kind="ExternalInput")
    wwq = nc.dram_tensor("wwq", [128, 640], cdt, kind="ExternalInput")
    wwr = nc.dram_tensor("wwr", [128, WWR], cdt, kind="ExternalInput")
    mbb = nc.dram_tensor("mbb", [128, 2], F32, kind="ExternalInput")
    vout = nc.dram_tensor("v", [T_LOC, D], _CDT if os.environ.get("ATTN_VOUT", "bf16") == "bf16" else F32, kind="ExternalOutput")
    with tile.TileContext(nc) as tc:
        _emit(nc, tc, cdt, xt, xe, wwq, wwr, mbb, vout)
    nc.compile()
    return nc


# ---------------- host-side prep ----------------

def prep_core_inputs(x, Wt, Wx, bh, Wa, ba, core, cdt=_CDT):
    ndt = _np_dt(cdt)
    xdt = _np_dt(_XDT)
    b, half = core // 2, core % 2
    t0 = half * T_LOC
    lo, hi = t0 - 32, t0 + 544
    pad_lo, pad_hi = max(0, -lo), max(0, hi - T)
    xs = x[b, max(0, lo):min(T, hi), :]
    x_halo = np.pad(xs, ((pad_lo, pad_hi), (0, 0)))     # [576, 512]

    # xt: [128, 4*576], chunk c = x_halo[:, 128c:128c+128].T
    xt = np.empty((128, 4 * HALO), np.float32)
    for c in range(4):
        xt[:, HALO * c:HALO * (c + 1)] = x_halo[:, 128 * c:128 * c + 128].T
    # xe: [128, 8*513], block m = rows [64m, 64m+128) with ones column
    xe_rows = np.concatenate(
        [x_halo, np.ones((HALO, 1), np.float32)], 1)    # [576, 513]
    xe = np.empty((128, 8 * 513), np.float32)
    for m in range(NBLK):
        xe[:, 513 * m:513 * (m + 1)] = xe_rows[64 * m:64 * m + 128, :]
    # wwq: [128, 640] = Wt tiled x4 [512] | Wx [128]
    wwq = np.zeros((128, 640), np.float32)
    for c in range(4):
        wwq[:, 128 * c:128 * c + 128] = np.tile(Wt[128 * c:128 * c + 128, :],
                                                (1, 4))
        wwq[:, 512 + 32 * c:512 + 32 * c + 32] = Wx[128 * c:128 * c + 128, :]
    # wwr: [128, 443] = wa_wide[124] | sh[255] | Rlo[32] | Rhi[32]
    wwr = np.zeros((128, WWR), np.float32)
    for g in range(4):
        wwr[32 * g:32 * g + 32, WA0 + 60 + g] = Wa[:, 0]
    kk = np.arange(128)
    wwr[kk, SH0 + kk + 127] = 1.0
    # edge-mask factors: E[d', t] += -30 where j = t0 + t + d' - 32 invalid.
    # left edge (t0 == 0):  invalid iff t + d' < 32  (d' = k in [0,32))
    # right edge (t0+512 == T): invalid iff t + d' > 543 (d' = k+32)
    ks = np.arange(32)[:, None]
    ts = np.arange(32)[None, :]
    if t0 == 0:
        wwr[0:32, RLO0:RLO0 + 32] = np.where(ts < 32 - ks, -30.0, 0.0)
    if t0 + T_LOC == T:
        wwr[0:32, RHI0:RHI0 + 32] = np.where((480 + ts) + (ks + 32) > 543,
                                             -30.0, 0.0)
    # mbb: [128, 2] = ba (rows 0-63) | bh4
    mbb = np.zeros((128, 2), np.float32)
    mbb[0:64, 0] = float(np.asarray(ba).reshape(-1)[0])
    mbb[:, 1] = np.tile(np.asarray(bh, np.float32), 4)

    return {
        "xt": xt.astype(xdt),
        "xe": xe.astype(ndt),
        "wwq": wwq.astype(ndt),
        "wwr": wwr.astype(ndt),
        "mbb": mbb,
    }


_NC_CACHE = {}


def _get_nc(cdt=_CDT):
    key = str(cdt)
    if key not in _NC_CACHE:
        _NC_CACHE[key] = build_nc(cdt)
    return _NC_CACHE[key]


def kernel(x, Wt, Wx, bh, Wa, ba, _trace=False):
    x = np.asarray(x, np.float32)
    Wt = np.asarray(Wt, np.float32)
    Wx = np.asarray(Wx, np.float32)
    bh = np.asarray(bh, np.float32)
    Wa = np.asarray(Wa, np.float32)
    ba = np.asarray(ba, np.float32)
    nc = _get_nc()
    in_maps = [prep_core_inputs(x, Wt, Wx, bh, Wa, ba, c)
               for c in range(NCORES)]
    res = run_bass_kernel_spmd(nc, in_maps, core_ids=list(range(NCORES)),
                               trace=_trace)
    out = np.empty((B, T, D), np.float32)
    for c in range(NCORES):
        b, half = c // 2, c % 2
        out[b, half * T_LOC:(half + 1) * T_LOC, :] = np.asarray(
            res.results[c]["v"], np.float32)
    if _trace:
        return out, res
    return out


# revision 22
# speedup vs baseline: 1.2027x; 1.1424x over previous
"""Trainium2 Bass kernel: banded additive attention (window 64).

reference semantics (B=4, T=1024, D=512, U=32, WIDTH=64):
  q = x @ Wt ; k = x @ Wx
  e[b,t,j] = exp(Wa . tanh(q[b,t]+k[b,j]+bh) + ba) for j in [t-32, t+31]
  a = e / (sum_j e + 1e-7) ; v = a @ x

Sharding: 8 NeuronCores = (batch b, T-half). Each core computes 512 query
rows with a 32-row halo; weights replicated. Inputs are packed/cast on the
host into SBUF-shaped blocks so each needs a single DMA.

Per-core pipeline (Tile-scheduled), hand-interleaved in two 256-query
halves so the exp/shear/value/output tail of half 0 hides under the
scoring of half 1:
  0. PE warmup chain keeps the tensor engine p-state ramped through the
     input-DMA wait; x and the q/k weights ship as fp8e4m3 to halve the
     critical input DMA.
  1. PE projections: kT split [0:322]/[322:576] (contract D in 4 chunks vs
     host-pretransposed xT), q4 in column halves; shifted-replicate
     matmuls (lhsT = slices of a 0/1 band constant) build the diagonal
     layout k4[32g+u, c] = kT[u, c+g]; PSUM->SBUF copies split so half-0
     scoring starts as early as possible.
  2. Per half: 4x (DVE add -> ACT tanh -> PE matmuls with sliced "wide Wa"
     lhsT accumulating 64 diagonals into E[64,256] PSUM).
  3. ACT exp(+ba) -> B0 in sigmaA order; radix-8 shear butterfly (shift
     matmuls + column-permute copies) -> banded B.
  4. Value: per 64-row block, B-block.T @ xe-block with a ones-column so
     the softmax denominator falls out of the same matmul; per-partition
     reciprocal scale; per-128-row output DMA.
"""
import os
import sys

sys.path.insert(0, "/opt/trn_rl_repo")

import numpy as np
import ml_dtypes  # noqa: E402
import concourse.bass as bass  # noqa: E402
import concourse.mybir as mybir  # noqa: E402
from concourse import bacc, tile  # noqa: E402
from concourse.ap import AP  # noqa: E402
from concourse.bass_utils import run_bass_kernel_spmd  # noqa: E402

F32 = mybir.dt.float32
BF16 = mybir.dt.bfloat16
FP8 = mybir.dt.float8e4
ActFn = mybir.ActivationFunctionType

B, T, D, U = 4, 1024, 512, 32
WIDTH = 64
EPS = 1e-7
T_LOC = 512
HALF = 256
HALO = 576
NBLK = 8
NCORES = 8

_CDT = BF16 if os.environ.get("ATTN_CDT", "bf16") == "bf16" else F32
_XDT = FP8 if os.environ.get("ATTN_XDT", "fp8") == "fp8" else _CDT
_NWARM = int(os.environ.get("ATTN_NWARM", "12"))

# kT column split points: k4[:,0:280) needs kT[:,0:283); score batches
# b1+b2 of half 0 read k4 cols [0:280) only
KTS1 = 283
KTS = 322
K4S1 = 280
K4S = 319

# wwr layout: wa_wide[124] | sh[255] | rlo[32] | rhi[32]
WA0, SH0, RLO0, RHI0, WWR = 0, 124, 379, 411, 443

# score batches per half: (slice0, nslices); h1 descends so the last
# E-matmul group (which gates exp) is small
BATCHES = [(0, 2), (2, 4), (6, 5), (11, 5)]
BATCHES1 = [(0, 5), (5, 5), (10, 5), (15, 1)]


def _np_dt(dt_):
    if dt_ == FP8:
        return ml_dtypes.float8_e4m3
    return ml_dtypes.bfloat16 if dt_ == BF16 else np.float32


def _emit(nc, tc, cdt, xt, xe, wwq, wwr, mbb, vout):
    from contextlib import ExitStack
    ctx = ExitStack()
    with ctx:
        cpool = ctx.enter_context(tc.tile_pool(name="consts", bufs=1))
        work = ctx.enter_context(tc.tile_pool(name="work", bufs=1))
        tpool = ctx.enter_context(tc.tile_pool(name="tanh", bufs=2))
        spool = ctx.enter_context(tc.tile_pool(name="shear", bufs=2))
        opool = ctx.enter_context(tc.tile_pool(name="outs", bufs=4))
        xdt = xt.dtype

        # ---------- Phase 0: ACT table preload + DMA loads + PE warmup -----
        dummy = cpool.tile([1, 1], F32, tag="dummy")
        nc.vector.memset(dummy[:], 0.0)
        nc.scalar.activation(dummy[:], dummy[:], ActFn.Exp)

        wwq_sb = cpool.tile([128, 640], cdt, tag="wwq_sb")
        nc.sync.dma_start(wwq_sb[:], wwq[:])
        xt_sb = cpool.tile([128, 4 * HALO], xdt, tag="xt_sb")
        nc.gpsimd.dma_start(xt_sb[:, 0:2 * HALO], xt[:, 0:2 * HALO])
        nc.sync.dma_start(xt_sb[:, 2 * HALO:], xt[:, 2 * HALO:])
        mbb_sb = cpool.tile([128, 2], F32, tag="mbb_sb")
        nc.sync.dma_start(mbb_sb[:], mbb[:])
        # wwr rides the Pool queue between xt-c01 and xe so the big xe
        # transfer cannot jump ahead of it on the DMA device
        wwr_sb = cpool.tile([128, WWR], cdt, tag="wwr_sb")
        nc.gpsimd.dma_start(wwr_sb[:], wwr[:])
        xe_all = cpool.tile([128, 8 * 513], cdt, tag="xe_all")
        nc.gpsimd.dma_start(xe_all[:], xe[:])

        def xt_chunk(c, lo, hi):
            return xt_sb[:, HALO * c + lo:HALO * c + hi]

        wa_sb = wwr_sb[:, WA0:WA0 + 124]
        sh_sb = wwr_sb[:, SH0:SH0 + 255]
        rlo_sb = wwr_sb[0:32, RLO0:RLO0 + 32]
        rhi_sb = wwr_sb[0:32, RHI0:RHI0 + 32]
        ba_sb = mbb_sb[0:64, 0:1]
        bh4_sb = mbb_sb[:, 1:2]

        # PE warmup: keep the tensor engine continuously busy through the
        # input-DMA wait so the p-state ramp is done before real matmuls.
        warm = cpool.tile([128, 256], cdt, tag="warm")
        nc.vector.memset(warm[:], 0.0)
        ones64 = cpool.tile([64, 1], cdt, tag="ones64")
        nc.vector.memset(ones64[:], 1.0)
        wstack = ExitStack()
        wps = wstack.enter_context(tc.tile_pool(name="wps", bufs=1,
                                                space="PSUM"))
        wp = wps.tile([128, 256], F32, tag="wp")
        for _ in range(_NWARM):
            nc.tensor.matmul(wp[:], warm[:, 0:128], warm[:],
                             start=True, stop=True)
        wstack.close()

        # ---------- Phase 1: projections (critical chain to first tanh) ---
        estack = ExitStack()
        ppe = estack.enter_context(tc.tile_pool(name="ppe", bufs=2,
                                                space="PSUM"))
        pstack = ExitStack()
        pp1 = pstack.enter_context(tc.tile_pool(name="pp1", bufs=1,
                                                space="PSUM"))
        kTa1_ps = pp1.tile([U, KTS1], F32, tag="kTa1")       # kT[:, 0:283]
        kTa2_ps = pp1.tile([U, KTS - KTS1], F32, tag="kTa2")  # [283:322]
        kTb_ps = pp1.tile([U, HALO - KTS], F32, tag="kTb")   # kT[:, 322:576]
        q4_ps = pp1.tile([128, T_LOC], F32, tag="q4_ps")
        k4_ps = pp1.tile([128, 1024], F32, tag="k4_ps")
        kT_sb = work.tile([U, HALO], cdt, tag="kT_sb")
        q4s = work.tile([128, T_LOC], cdt, tag="q4s")
        k4 = work.tile([128, HALO], cdt, tag="k4")

        def wx(c):
            return wwq_sb[:, 512 + 32 * c:512 + 32 * c + 32]

        # chunk-pair interleave: c0/c1 arrive first (Pool queue), c2/c3 on
        # the SP queue land ~400ns later
        # tiny kTa2 matmuls lead each post-DMA-wait group: the p-state
        # model runs the first ~2 matmuls after a wait at mid clock, so
        # let the 39-column ones absorb that
        for c in range(2):
            nc.tensor.matmul(kTa2_ps[:], wx(c), xt_chunk(c, KTS1, KTS),
                             start=(c == 0), stop=False)
        for c in range(2):
            nc.tensor.matmul(kTa1_ps[:], wx(c), xt_chunk(c, 0, KTS1),
                             start=(c == 0), stop=False)
        for c in range(2):
            nc.tensor.matmul(q4_ps[:, 0:HALF],
                             wwq_sb[:, 128 * c:128 * c + 128],
                             xt_chunk(c, 32, 32 + HALF),
                             start=(c == 0), stop=False)
        for c in range(2, 4):
            nc.tensor.matmul(kTa2_ps[:], wx(c), xt_chunk(c, KTS1, KTS),
                             start=False, stop=(c == 3))
        for c in range(2, 4):
            nc.tensor.matmul(kTa1_ps[:], wx(c), xt_chunk(c, 0, KTS1),
                             start=False, stop=(c == 3))
        for c in range(2, 4):
            nc.tensor.matmul(q4_ps[:, 0:HALF],
                             wwq_sb[:, 128 * c:128 * c + 128],
                             xt_chunk(c, 32, 32 + HALF),
                             start=False, stop=(c == 3))
        nc.vector.tensor_copy(kT_sb[:, 0:KTS1], kTa1_ps[:])    # DVE
        # k4a-c1: k4[32g+u, c] = kT[u, c+g] for c in [0:280)
        for g in range(4):
            lhsT = sh_sb[0:32, 127 - 32 * g:255 - 32 * g]
            nc.tensor.matmul(k4_ps[:, 0:K4S1],
                             lhsT, kT_sb[:, g:g + K4S1],
                             start=(g == 0), stop=(g == 3))
        for c in range(4):  # PE filler while DVE copies
            nc.tensor.matmul(kTb_ps[:], wx(c), xt_chunk(c, KTS, HALO),
                             start=(c == 0), stop=(c == 3))
        # q4s-h0 on ACT (idle pre-tanh) keeps DVE free for the k4 chain
        nc.scalar.activation(q4s[:, 0:HALF], q4_ps[:, 0:HALF],
                             ActFn.Identity, bias=bh4_sb)
        nc.scalar.copy(kT_sb[:, KTS:HALO], kTb_ps[:])          # ACT (idle)
        nc.vector.tensor_copy(k4[:, 0:K4S1], k4_ps[:, 0:K4S1])  # DVE
        for c in range(4):
            nc.tensor.matmul(q4_ps[:, HALF:T_LOC],
                             wwq_sb[:, 128 * c:128 * c + 128],
                             xt_chunk(c, 32 + HALF, 32 + T_LOC),
                             start=(c == 0), stop=(c == 3))
        nc.scalar.activation(q4s[:, HALF:T_LOC], q4_ps[:, HALF:T_LOC],
                             ActFn.Identity, bias=bh4_sb)

        # ---------- Phases 2-4, hand-interleaved over halves ----------
        E = [None, None]
        B0 = [None, None]
        S1 = [None, None]
        P1 = [None, None]
        P2 = [None, None]
        Bsb = [None, None]
        vp = {}

        def score_batch(h, s0, nsl):
            if E[h] is None:
                E[h] = ppe.tile([64, HALF], F32, tag="E", name="E")
            tin = tpool.tile([128, HALF * nsl], cdt, tag="tin")
            k4ap = AP(k4[:].tensor, 4 * s0 + HALF * h,
                      [[HALO, 128], [4, nsl], [1, HALF]])
            q4ap = AP(q4s[:].tensor, HALF * h,
                      [[T_LOC, 128], [0, nsl], [1, HALF]])
            nc.vector.tensor_add(
                tin[:].rearrange("p (a t) -> p a t", a=nsl), q4ap, k4ap)
            tout = tpool.tile([128, HALF * nsl], cdt, tag="tout")
            nc.scalar.activation(tout[:], tin[:], ActFn.Tanh)
            for j in range(nsl):
                r = s0 + j
                nc.tensor.matmul(E[h][:],
                                 wa_sb[:, 60 - 4 * r:124 - 4 * r],
                                 tout[:, HALF * j:HALF * j + HALF],
                                 start=(r == 0), stop=False)

        def edge(h):
            # E += -30 on invalid j (zero matrices on interior cores)
            if h == 0:
                nc.tensor.matmul(E[0][:, 0:32], sh_sb[0:32, 127:191],
                                 rlo_sb, start=False, stop=True)
            else:
                nc.tensor.matmul(E[1][:, 224:256], sh_sb[0:32, 95:159],
                                 rhi_sb, start=False, stop=True)

        def exp_phase(h):
            B0[h] = spool.tile([64, HALF], cdt, tag="B0", name="B0")
            nc.scalar.activation(B0[h][:], E[h][:], ActFn.Exp, bias=ba_sb)

        def shear_mm(h):
            # one-stage shear: column t = 64m+8a+b needs shift s = 8a+b.
            # 64 tiny matmuls, one per s; rhs = B0 cols {64m + s} (stride 64)
            P1[h] = pps.tile([128, HALF], F32, tag="P1", name="P1")
            if os.environ.get("ATTN_SHEAR1", "1") == "1":
                for s in range(64):
                    rhs = AP(B0[h][:].tensor, s, [[HALF, 64], [64, 4]])
                    nc.tensor.matmul(P1[h][:, 4 * s:4 * s + 4],
                                     sh_sb[0:64, 127 - s:255 - s], rhs,
                                     start=True, stop=True)
            else:
                for s in range(64):
                    for i in range(4):
                        nc.tensor.matmul(
                            P1[h][:, 4 * s + i:4 * s + i + 1],
                            sh_sb[0:64, 127 - s:255 - s],
                            B0[h][:, 64 * i + s:64 * i + s + 1],
                            start=True, stop=True)

        def shear_copy(h, half2, eng="dve"):
            # Bsb col 64m+8a+b <- P1 col 4(8a+b)+m, iter (a,b,m); emitted in
            # two m-pair chunks so the first value group starts earlier
            if half2 == 0:
                Bsb[h] = spool.tile([128, HALF], cdt, tag="Bsb", name="Bsb")
            bsb_out = AP(Bsb[h][:].tensor, 128 * half2,
                         [[HALF, 128], [8, 8], [1, 8], [64, 2]])
            p_in = AP(P1[h][:].tensor, 2 * half2,
                      [[HALF, 128], [32, 8], [4, 8], [1, 2]])
            if eng == "act":
                nc.scalar.copy(bsb_out, p_in)
            else:
                nc.vector.tensor_copy(bsb_out, p_in)

        def value_mm(h, mp):
            t = vpool.tile([128, 512], F32, tag="vp", name="vp")
            vp[(h, mp)] = t
            for h2 in range(2):
                m2 = 2 * mp + h2
                m = 4 * h + m2
                nc.tensor.matmul(t[64 * h2:64 * h2 + 64, :],
                                 Bsb[h][:, 64 * m2:64 * m2 + 64],
                                 xe_all[:, 513 * m:513 * m + 512],
                                 start=True, stop=True)

        rcols = {}

        def denom(h):
            # softmax denominators straight from B0 (plain t-order): one
            # ones-contraction matmul per 128-query group, then reciprocal.
            # EPS dropped: s >= 64*exp(-~5) makes 1e-7 negligible
            for mp in range(2):
                col = 2 * h + mp
                nc.tensor.matmul(spsum[:, col:col + 1],
                                 B0[h][:, 128 * mp:128 * mp + 128],
                                 ones64[:], start=True, stop=True)
                rcol = opool.tile([128, 1], F32, tag="rcol", name="rcol")
                nc.vector.reciprocal(rcol[:], spsum[:, col:col + 1])
                rcols[(h, mp)] = rcol

        def value_scale(h, mp, scale_eng, vsb, col):
            t = vp[(h, mp)]
            rcol = rcols[(h, mp)]
            if scale_eng == "split":
                nc.scalar.activation(vsb[:, col:col + HALF], t[:, 0:HALF],
                                     ActFn.Copy, scale=rcol[:])
                nc.vector.tensor_scalar_mul(vsb[:, col + HALF:col + D],
                                            t[:, HALF:D], rcol[:])
            elif scale_eng == "act":
                nc.scalar.activation(vsb[:, col:col + D], t[:, 0:512],
                                     ActFn.Copy, scale=rcol[:])
            else:
                nc.vector.tensor_scalar_mul(vsb[:, col:col + D],
                                            t[:, 0:512], rcol[:])

        def value_out(h, mp, scale_eng):
            vsb = opool.tile([128, D], vout.dtype, tag="vsb", name="vsb")
            value_scale(h, mp, scale_eng, vsb, 0)
            row = 256 * h + 128 * mp
            nc.sync.dma_start(vout[row:row + 128, :], vsb[:])

        # --- interleaved emission ---
        score_batch(0, *BATCHES[0])
        score_batch(0, *BATCHES[1])
        nc.vector.tensor_copy(kT_sb[:, KTS1:KTS], kTa2_ps[:])
        for g in range(4):  # k4a-c2 shifts (need kTa2 copy above)
            lhsT = sh_sb[0:32, 127 - 32 * g:255 - 32 * g]
            nc.tensor.matmul(k4_ps[:, K4S1:K4S],
                             lhsT, kT_sb[:, K4S1 + g:K4S + g],
                             start=(g == 0), stop=(g == 3))
        nc.vector.tensor_copy(k4[:, K4S1:K4S], k4_ps[:, K4S1:K4S])
        score_batch(0, *BATCHES[2])
        for g in range(4):  # k4b shifts (need kTa2 + kTb copies);
            lhsT = sh_sb[0:32, 127 - 32 * g:255 - 32 * g]
            nc.tensor.matmul(k4_ps[:, K4S:512],  # psum-bank split at 512
                             lhsT, kT_sb[:, K4S + g:512 + g],
                             start=(g == 0), stop=(g == 3))
        for g in range(4):
            lhsT = sh_sb[0:32, 127 - 32 * g:255 - 32 * g]
            nc.tensor.matmul(k4_ps[:, 512:573],
                             lhsT, kT_sb[:, 512 + g:573 + g],
                             start=(g == 0), stop=(g == 3))
        score_batch(0, *BATCHES[3])
        nc.vector.tensor_copy(k4[:, K4S:573], k4_ps[:, K4S:573])
        edge(0)
        pstack.close()
        pps = estack.enter_context(tc.tile_pool(name="pps", bufs=1,
                                                space="PSUM"))
        vpool = estack.enter_context(tc.tile_pool(name="vpsum", bufs=2,
                                                  space="PSUM"))
        spool_ps = estack.enter_context(tc.tile_pool(name="sps", bufs=1,
                                                     space="PSUM"))
        spsum = spool_ps.tile([128, 4], F32, tag="spsum")

        score_batch(1, *BATCHES1[0])
        exp_phase(0)
        denom(0)
        shear_mm(0)
        score_batch(1, *BATCHES1[1])
        shear_copy(0, 0)
        shear_copy(0, 1)
        score_batch(1, *BATCHES1[2])
        score_batch(1, *BATCHES1[3])
        edge(1)
        value_mm(0, 0)
        value_out(0, 0, "dve")
        value_mm(0, 1)
        value_out(0, 1, "act")
        exp_phase(1)
        denom(1)
        shear_mm(1)
        shear_copy(1, 0)
        value_mm(1, 0)
        shear_copy(1, 1, "act")
        value_mm(1, 1)
        value_out(1, 0, "dve")
        value_out(1, 1, "split")
        estack.close()


def build_nc(cdt=_CDT):
    nc = bacc.Bacc("TRN2", target_bir_lowering=False)
    xt = nc.dram_tensor("xt", [128, 4 * HALO], _XDT, kind="ExternalInput")
    xe = nc.dram_tensor("xe", [128, 8 * 513], cdt, kind="ExternalInput")
    wwq = nc.dram_tensor("wwq", [128, 640], cdt, kind="ExternalInput")
    wwr = nc.dram_tensor("wwr", [128, WWR], cdt, kind="ExternalInput")
    mbb = nc.dram_tensor("mbb", [128, 2], F32, kind="ExternalInput")
    vout = nc.dram_tensor("v", [T_LOC, D], _CDT if os.environ.get("ATTN_VOUT", "bf16") == "bf16" else F32, kind="ExternalOutput")
    with tile.TileContext(nc) as tc:
        _emit(nc, tc, cdt, xt, xe, wwq, wwr, mbb, vout)
    nc.compile()
    return nc


# ---------------- host-side prep ----------------

def prep_core_inputs(x, Wt, Wx, bh, Wa, ba, core, cdt=_CDT):
    ndt = _np_dt(cdt)
    xdt = _np_dt(_XDT)
    b, half = core // 2, core % 2
    t0 = half * T_LOC
    lo, hi = t0 - 32, t0 + 544
    pad_lo, pad_hi = max(0, -lo), max(0, hi - T)
    xs = x[b, max(0, lo):min(T, hi), :]
    x_halo = np.pad(xs, ((pad_lo, pad_hi), (0, 0)))     # [576, 512]

    # xt: [128, 4*576], chunk c = x_halo[:, 128c:128c+128].T
    xt = np.empty((128, 4 * HALO), np.float32)
    for c in range(4):
        xt[:, HALO * c:HALO * (c + 1)] = x_halo[:, 128 * c:128 * c + 128].T
    # xe: [128, 8*513], block m = rows [64m, 64m+128) with ones column
    xe_rows = np.concatenate(
        [x_halo, np.ones((HALO, 1), np.float32)], 1)    # [576, 513]
    xe = np.empty((128, 8 * 513), np.float32)
    for m in range(NBLK):
        xe[:, 513 * m:513 * (m + 1)] = xe_rows[64 * m:64 * m + 128, :]
    # wwq: [128, 640] = Wt tiled x4 [512] | Wx [128]
    wwq = np.zeros((128, 640), np.float32)
    for c in range(4):
        wwq[:, 128 * c:128 * c + 128] = np.tile(Wt[128 * c:128 * c + 128, :],
                                                (1, 4))
        wwq[:, 512 + 32 * c:512 + 32 * c + 32] = Wx[128 * c:128 * c + 128, :]
    # wwr: [128, 443] = wa_wide[124] | sh[255] | Rlo[32] | Rhi[32]
    wwr = np.zeros((128, WWR), np.float32)
    for g in range(4):
        wwr[32 * g:32 * g + 32, WA0 + 60 + g] = Wa[:, 0]
    kk = np.arange(128)
    wwr[kk, SH0 + kk + 127] = 1.0
    # edge-mask factors: E[d', t] += -30 where j = t0 + t + d' - 32 invalid.
    # left edge (t0 == 0):  invalid iff t + d' < 32  (d' = k in [0,32))
    # right edge (t0+512 == T): invalid iff t + d' > 543 (d' = k+32)
    ks = np.arange(32)[:, None]
    ts = np.arange(32)[None, :]
    if t0 == 0:
        wwr[0:32, RLO0:RLO0 + 32] = np.where(ts < 32 - ks, -30.0, 0.0)
    if t0 + T_LOC == T:
        wwr[0:32, RHI0:RHI0 + 32] = np.where((480 + ts) + (ks + 32) > 543,
                                             -30.0, 0.0)
    # mbb: [128, 2] = ba (rows 0-63) | bh4
    mbb = np.zeros((128, 2), np.float32)
    mbb[0:64, 0] = float(np.asarray(ba).reshape(-1)[0])
    mbb[:, 1] = np.tile(np.asarray(bh, np.float32), 4)

    return {
        "xt": xt.astype(xdt),
        "xe": xe.astype(ndt),
        "wwq": wwq.astype(ndt),
        "wwr": wwr.astype(ndt),
        "mbb": mbb,
    }


_NC_CACHE = {}


def _get_nc(cdt=_CDT):
    key = str(cdt)
    if key not in _NC_CACHE:
        _NC_CACHE[key] = build_nc(cdt)
    return _NC_CACHE[key]


def kernel(x, Wt, Wx, bh, Wa, ba, _trace=False):
    x = np.asarray(x, np.float32)
    Wt = np.asarray(Wt, np.float32)
    Wx = np.asarray(Wx, np.float32)
    bh = np.asarray(bh, np.float32)
    Wa = np.asarray(Wa, np.float32)
    ba = np.asarray(ba, np.float32)
    nc = _get_nc()
    in_maps = [prep_core_inputs(x, Wt, Wx, bh, Wa, ba, c)
               for c in range(NCORES)]
    res = run_bass_kernel_spmd(nc, in_maps, core_ids=list(range(NCORES)),
                               trace=_trace)
    out = np.empty((B, T, D), np.float32)
    for c in range(NCORES):
        b, half = c // 2, c % 2
        out[b, half * T_LOC:(half + 1) * T_LOC, :] = np.asarray(
            res.results[c]["v"], np.float32)
    if _trace:
        return out, res
    return out


# revision 30
# speedup vs baseline: 1.2128x; 1.0084x over previous
"""Trainium2 Bass kernel: banded additive attention (window 64).

reference semantics (B=4, T=1024, D=512, U=32, WIDTH=64):
  q = x @ Wt ; k = x @ Wx
  e[b,t,j] = exp(Wa . tanh(q[b,t]+k[b,j]+bh) + ba) for j in [t-32, t+31]
  a = e / (sum_j e + 1e-7) ; v = a @ x

Sharding: 8 NeuronCores = (batch b, T-half). Each core computes 512 query
rows with a 32-row halo; weights replicated. Inputs are packed/cast on the
host into SBUF-shaped blocks so each needs a single DMA.

Per-core pipeline (Tile-scheduled), hand-interleaved in two 256-query
halves so the exp/shear/value/output tail of half 0 hides under the
scoring of half 1:
  0. PE warmup chain keeps the tensor engine p-state ramped through the
     input-DMA wait; x and the q/k weights ship as fp8e4m3 to halve the
     critical input DMA.
  1. PE projections: kT split [0:322]/[322:576] (contract D in 4 chunks vs
     host-pretransposed xT), q4 in column halves; shifted-replicate
     matmuls (lhsT = slices of a 0/1 band constant) build the diagonal
     layout k4[32g+u, c] = kT[u, c+g]; PSUM->SBUF copies split so half-0
     scoring starts as early as possible.
  2. Per half: 4x (DVE add -> ACT tanh -> PE matmuls with sliced "wide Wa"
     lhsT accumulating 64 diagonals into E[64,256] PSUM).
  3. ACT exp(+ba) -> B0 in sigmaA order; radix-8 shear butterfly (shift
     matmuls + column-permute copies) -> banded B.
  4. Value: per 64-row block, B-block.T @ xe-block with a ones-column so
     the softmax denominator falls out of the same matmul; per-partition
     reciprocal scale; per-128-row output DMA.
"""
import os
import sys

sys.path.insert(0, "/opt/trn_rl_repo")

import numpy as np
import ml_dtypes  # noqa: E402
import concourse.bass as bass  # noqa: E402
import concourse.mybir as mybir  # noqa: E402
from concourse import bacc, tile  # noqa: E402
from concourse.ap import AP  # noqa: E402
from concourse.bass_utils import run_bass_kernel_spmd  # noqa: E402

F32 = mybir.dt.float32
BF16 = mybir.dt.bfloat16
FP8 = mybir.dt.float8e4
ActFn = mybir.ActivationFunctionType

B, T, D, U = 4, 1024, 512, 32
WIDTH = 64
EPS = 1e-7
T_LOC = 512
HALF = 256
HALO = 576
NBLK = 8
NCORES = 8

_CDT = BF16 if os.environ.get("ATTN_CDT", "bf16") == "bf16" else F32
_XDT = FP8 if os.environ.get("ATTN_XDT", "fp8") == "fp8" else _CDT
_NWARM = int(os.environ.get("ATTN_NWARM", "12"))

# kT column split points: k4[:,0:280) needs kT[:,0:283); score batches
# b1+b2 of half 0 read k4 cols [0:280) only
KTS1 = 291
KTS = 322
K4S1 = 288
K4S = 319

# wwr layout: wa_wide[124] | sh[255] | rlo[32] | rhi[32]
WA0, SH0, RLO0, RHI0, WWR = 0, 124, 379, 411, 443

# score batches per half: (slice0, nslices); h1 descends so the last
# E-matmul group (which gates exp) is small
BATCHES = [(0, 2), (2, 4), (6, 5), (11, 5)]
BATCHES1 = [(0, 5), (5, 5), (10, 5), (15, 1)]


def _np_dt(dt_):
    if dt_ == FP8:
        return ml_dtypes.float8_e4m3
    return ml_dtypes.bfloat16 if dt_ == BF16 else np.float32


def _emit(nc, tc, cdt, xt, xe, wwq, wwr, mbb, vout):
    from contextlib import ExitStack
    ctx = ExitStack()
    with ctx:
        cpool = ctx.enter_context(tc.tile_pool(name="consts", bufs=1))
        work = ctx.enter_context(tc.tile_pool(name="work", bufs=1))
        tpool = ctx.enter_context(tc.tile_pool(name="tanh", bufs=2))
        spool = ctx.enter_context(tc.tile_pool(name="shear", bufs=2))
        opool = ctx.enter_context(tc.tile_pool(name="outs", bufs=4))
        xdt = xt.dtype

        # ---------- Phase 0: ACT table preload + DMA loads + PE warmup -----
        dummy = cpool.tile([1, 1], F32, tag="dummy")
        nc.vector.memset(dummy[:], 0.0)
        nc.scalar.activation(dummy[:], dummy[:], ActFn.Exp)

        # xt leads on the SP/HWDGE queue (its transfer is the critical
        # gate); the weights ride the Pool queue with the tiny Wx columns
        # first, so every transfer lands just before its first consumer.
        # xe goes last -- its descgen serializes behind the others on Pool
        # so it cannot jump ahead on the DMA device.
        xt_sb = cpool.tile([128, 4 * HALO], xdt, tag="xt_sb")
        nc.sync.dma_start(xt_sb[:], xt[:])
        wwq_sb = cpool.tile([128, 640], cdt, tag="wwq_sb")
        nc.gpsimd.dma_start(wwq_sb[:], wwq[:])
        mbb_sb = cpool.tile([128, 2], F32, tag="mbb_sb")
        nc.sync.dma_start(mbb_sb[:], mbb[:])
        wwr_sb = cpool.tile([128, WWR], cdt, tag="wwr_sb")
        nc.gpsimd.dma_start(wwr_sb[:], wwr[:])
        xe_all = cpool.tile([128, 8 * 513], cdt, tag="xe_all")
        nc.gpsimd.dma_start(xe_all[:], xe[:])

        def xt_chunk(c, lo, hi):
            return xt_sb[:, HALO * c + lo:HALO * c + hi]

        wa_sb = wwr_sb[:, WA0:WA0 + 124]
        sh_sb = wwr_sb[:, SH0:SH0 + 255]
        rlo_sb = wwr_sb[0:32, RLO0:RLO0 + 32]
        rhi_sb = wwr_sb[0:32, RHI0:RHI0 + 32]
        ba_sb = mbb_sb[0:64, 0:1]
        bh4_sb = mbb_sb[:, 1:2]

        # PE warmup: keep the tensor engine continuously busy through the
        # input-DMA wait so the p-state ramp is done before real matmuls.
        warm = cpool.tile([128, 256], cdt, tag="warm")
        nc.vector.memset(warm[:], 0.0)
        ones64 = cpool.tile([64, 1], cdt, tag="ones64")
        nc.vector.memset(ones64[:], 1.0)
        wstack = ExitStack()
        wps = wstack.enter_context(tc.tile_pool(name="wps", bufs=1,
                                                space="PSUM"))
        wp = wps.tile([128, 256], F32, tag="wp")
        for _ in range(_NWARM):
            nc.tensor.matmul(wp[:], warm[:, 0:128], warm[:],
                             start=True, stop=True)
        wstack.close()

        # ---------- Phase 1: projections (critical chain to first tanh) ---
        estack = ExitStack()
        ppe = estack.enter_context(tc.tile_pool(name="ppe", bufs=2,
                                                space="PSUM"))
        pstack = ExitStack()
        pp1 = pstack.enter_context(tc.tile_pool(name="pp1", bufs=1,
                                                space="PSUM"))
        kTa1_ps = pp1.tile([U, KTS1], F32, tag="kTa1")       # kT[:, 0:283]
        kTa2_ps = pp1.tile([U, KTS - KTS1], F32, tag="kTa2")  # [283:322]
        kTb_ps = pp1.tile([U, HALO - KTS], F32, tag="kTb")   # kT[:, 322:576]
        q4_ps = pp1.tile([128, T_LOC], F32, tag="q4_ps")
        k4_ps = pp1.tile([128, 1024], F32, tag="k4_ps")
        kT_sb = work.tile([U, HALO], cdt, tag="kT_sb")
        q4s = work.tile([128, T_LOC], cdt, tag="q4s")
        k4 = work.tile([128, HALO], cdt, tag="k4")

        def wx(c):
            return wwq_sb[:, 512 + 32 * c:512 + 32 * c + 32]

        # chunk-pair interleave: c0/c1 arrive first (Pool queue), c2/c3 on
        # the SP queue land ~400ns later
        # tiny kTa2 matmuls lead each post-DMA-wait group: the p-state
        # model runs the first ~2 matmuls after a wait at mid clock, so
        # let the 39-column ones absorb that
        for c in range(2):
            nc.tensor.matmul(kTa2_ps[:], wx(c), xt_chunk(c, KTS1, KTS),
                             start=(c == 0), stop=False)
        for c in range(2):
            nc.tensor.matmul(kTa1_ps[:], wx(c), xt_chunk(c, 0, KTS1),
                             start=(c == 0), stop=False)
        # kTa2-c2 absorbs the post-wait mid-clock penalty; kTa2's stop is
        # emitted last so its SBUF copy cannot jump the kTa1 copy in the
        # DVE wait queue.  q4 matmuls are emitted after k4a-c1 because they
        # wait on the late Wt columns and would clog the 4-deep PE wait
        # queue ahead of the critical shift matmuls.
        nc.tensor.matmul(kTa2_ps[:], wx(2), xt_chunk(2, KTS1, KTS),
                         start=False, stop=False)
        for c in range(2, 4):
            nc.tensor.matmul(kTa1_ps[:], wx(c), xt_chunk(c, 0, KTS1),
                             start=False, stop=(c == 3))
        nc.tensor.matmul(kTa2_ps[:], wx(3), xt_chunk(3, KTS1, KTS),
                         start=False, stop=True)
        nc.vector.tensor_copy(kT_sb[:, 0:KTS1], kTa1_ps[:])    # DVE
        # k4a-c1: k4[32g+u, c] = kT[u, c+g] for c in [0:288)
        for g in range(4):
            lhsT = sh_sb[0:32, 127 - 32 * g:255 - 32 * g]
            nc.tensor.matmul(k4_ps[:, 0:K4S1],
                             lhsT, kT_sb[:, g:g + K4S1],
                             start=(g == 0), stop=(g == 3))
        for c in range(4):
            nc.tensor.matmul(q4_ps[:, 0:HALF],
                             wwq_sb[:, 128 * c:128 * c + 128],
                             xt_chunk(c, 32, 32 + HALF),
                             start=(c == 0), stop=(c == 3))
        # q4s-h0 on ACT (idle pre-tanh) keeps DVE free for the k4 chain
        nc.scalar.activation(q4s[:, 0:HALF], q4_ps[:, 0:HALF],
                             ActFn.Identity, bias=bh4_sb)
        nc.vector.tensor_copy(k4[:, 0:K4S1], k4_ps[:, 0:K4S1])  # DVE
        for c in range(4):  # deferred projections fill PE during copies
            nc.tensor.matmul(kTb_ps[:], wx(c), xt_chunk(c, KTS, HALO),
                             start=(c == 0), stop=(c == 3))
        nc.scalar.copy(kT_sb[:, KTS:HALO], kTb_ps[:])          # ACT (idle)
        for c in range(4):
            nc.tensor.matmul(q4_ps[:, HALF:T_LOC],
                             wwq_sb[:, 128 * c:128 * c + 128],
                             xt_chunk(c, 32 + HALF, 32 + T_LOC),
                             start=(c == 0), stop=(c == 3))
        nc.scalar.activation(q4s[:, HALF:T_LOC], q4_ps[:, HALF:T_LOC],
                             ActFn.Identity, bias=bh4_sb)

        # ---------- Phases 2-4, hand-interleaved over halves ----------
        E = [None, None]
        B0 = [None, None]
        S1 = [None, None]
        P1 = [None, None]
        P2 = [None, None]
        Bsb = [None, None]
        vp = {}

        def score_batch(h, s0, nsl):
            if E[h] is None:
                E[h] = ppe.tile([64, HALF], F32, tag="E", name="E")
            tin = tpool.tile([128, HALF * nsl], cdt, tag="tin")
            k4ap = AP(k4[:].tensor, 4 * s0 + HALF * h,
                      [[HALO, 128], [4, nsl], [1, HALF]])
            q4ap = AP(q4s[:].tensor, HALF * h,
                      [[T_LOC, 128], [0, nsl], [1, HALF]])
            nc.vector.tensor_add(
                tin[:].rearrange("p (a t) -> p a t", a=nsl), q4ap, k4ap)
            tout = tpool.tile([128, HALF * nsl], cdt, tag="tout")
            nc.scalar.activation(tout[:], tin[:], ActFn.Tanh)
            for j in range(nsl):
                r = s0 + j
                nc.tensor.matmul(E[h][:],
                                 wa_sb[:, 60 - 4 * r:124 - 4 * r],
                                 tout[:, HALF * j:HALF * j + HALF],
                                 start=(r == 0), stop=False)

        def edge(h):
            # E += -30 on invalid j (zero matrices on interior cores)
            if h == 0:
                nc.tensor.matmul(E[0][:, 0:32], sh_sb[0:32, 127:191],
                                 rlo_sb, start=False, stop=True)
            else:
                nc.tensor.matmul(E[1][:, 224:256], sh_sb[0:32, 95:159],
                                 rhi_sb, start=False, stop=True)

        def exp_phase(h):
            B0[h] = spool.tile([64, HALF], cdt, tag="B0", name="B0")
            nc.scalar.activation(B0[h][:], E[h][:], ActFn.Exp, bias=ba_sb)

        def shear_mm(h):
            # one-stage shear: column t = 64m+8a+b needs shift s = 8a+b.
            # 64 tiny matmuls, one per s; rhs = B0 cols {64m + s} (stride 64)
            P1[h] = pps.tile([128, HALF], F32, tag="P1", name="P1")
            if os.environ.get("ATTN_SHEAR1", "1") == "1":
                for s in range(64):
                    rhs = AP(B0[h][:].tensor, s, [[HALF, 64], [64, 4]])
                    nc.tensor.matmul(P1[h][:, 4 * s:4 * s + 4],
                                     sh_sb[0:64, 127 - s:255 - s], rhs,
                                     start=True, stop=True)
            else:
                for s in range(64):
                    for i in range(4):
                        nc.tensor.matmul(
                            P1[h][:, 4 * s + i:4 * s + i + 1],
                            sh_sb[0:64, 127 - s:255 - s],
                            B0[h][:, 64 * i + s:64 * i + s + 1],
                            start=True, stop=True)

        def shear_copy(h, half2, eng="dve"):
            # Bsb col 64m+8a+b <- P1 col 4(8a+b)+m, iter (a,b,m); emitted in
            # two m-pair chunks so the first value group starts earlier
            if half2 == 0:
                Bsb[h] = spool.tile([128, HALF], cdt, tag="Bsb", name="Bsb")
            bsb_out = AP(Bsb[h][:].tensor, 128 * half2,
                         [[HALF, 128], [8, 8], [1, 8], [64, 2]])
            p_in = AP(P1[h][:].tensor, 2 * half2,
                      [[HALF, 128], [32, 8], [4, 8], [1, 2]])
            if eng == "act":
                nc.scalar.copy(bsb_out, p_in)
            else:
                nc.vector.tensor_copy(bsb_out, p_in)

        def value_mm(h, mp):
            t = vpool.tile([128, 512], F32, tag="vp", name="vp")
            vp[(h, mp)] = t
            for h2 in range(2):
                m2 = 2 * mp + h2
                m = 4 * h + m2
                nc.tensor.matmul(t[64 * h2:64 * h2 + 64, :],
                                 Bsb[h][:, 64 * m2:64 * m2 + 64],
                                 xe_all[:, 513 * m:513 * m + 512],
                                 start=True, stop=True)

        rcols = {}

        def denom(h):
            # softmax denominators straight from B0 (plain t-order): one
            # ones-contraction matmul per 128-query group, then reciprocal.
            # EPS dropped: s >= 64*exp(-~5) makes 1e-7 negligible
            for mp in range(2):
                col = 2 * h + mp
                nc.tensor.matmul(spsum[:, col:col + 1],
                                 B0[h][:, 128 * mp:128 * mp + 128],
                                 ones64[:], start=True, stop=True)
                rcol = opool.tile([128, 1], F32, tag="rcol", name="rcol")
                nc.vector.reciprocal(rcol[:], spsum[:, col:col + 1])
                rcols[(h, mp)] = rcol

        def value_scale(h, mp, scale_eng, vsb, col):
            t = vp[(h, mp)]
            rcol = rcols[(h, mp)]
            if scale_eng == "split":
                nc.scalar.activation(vsb[:, col:col + HALF], t[:, 0:HALF],
                                     ActFn.Copy, scale=rcol[:])
                nc.vector.tensor_scalar_mul(vsb[:, col + HALF:col + D],
                                            t[:, HALF:D], rcol[:])
            elif scale_eng == "act":
                nc.scalar.activation(vsb[:, col:col + D], t[:, 0:512],
                                     ActFn.Copy, scale=rcol[:])
            else:
                nc.vector.tensor_scalar_mul(vsb[:, col:col + D],
                                            t[:, 0:512], rcol[:])

        def value_out(h, mp, scale_eng):
            vsb = opool.tile([128, D], vout.dtype, tag="vsb", name="vsb")
            value_scale(h, mp, scale_eng, vsb, 0)
            row = 256 * h + 128 * mp
            nc.sync.dma_start(vout[row:row + 128, :], vsb[:])

        # --- interleaved emission ---
        score_batch(0, *BATCHES[0])
        score_batch(0, *BATCHES[1])
        nc.vector.tensor_copy(kT_sb[:, KTS1:KTS], kTa2_ps[:])
        for g in range(4):  # k4a-c2 shifts (need kTa2 copy above)
            lhsT = sh_sb[0:32, 127 - 32 * g:255 - 32 * g]
            nc.tensor.matmul(k4_ps[:, K4S1:K4S],
                             lhsT, kT_sb[:, K4S1 + g:K4S + g],
                             start=(g == 0), stop=(g == 3))
        nc.vector.tensor_copy(k4[:, K4S1:K4S], k4_ps[:, K4S1:K4S])
        score_batch(0, *BATCHES[2])
        for g in range(4):  # k4b shifts (need kTa2 + kTb copies);
            lhsT = sh_sb[0:32, 127 - 32 * g:255 - 32 * g]
            nc.tensor.matmul(k4_ps[:, K4S:512],  # psum-bank split at 512
                             lhsT, kT_sb[:, K4S + g:512 + g],
                             start=(g == 0), stop=(g == 3))
        for g in range(4):
            lhsT = sh_sb[0:32, 127 - 32 * g:255 - 32 * g]
            nc.tensor.matmul(k4_ps[:, 512:573],
                             lhsT, kT_sb[:, 512 + g:573 + g],
                             start=(g == 0), stop=(g == 3))
        score_batch(0, *BATCHES[3])
        nc.vector.tensor_copy(k4[:, K4S:573], k4_ps[:, K4S:573])
        edge(0)
        pstack.close()
        pps = estack.enter_context(tc.tile_pool(name="pps", bufs=1,
                                                space="PSUM"))
        vpool = estack.enter_context(tc.tile_pool(name="vpsum", bufs=2,
                                                  space="PSUM"))
        spool_ps = estack.enter_context(tc.tile_pool(name="sps", bufs=1,
                                                     space="PSUM"))
        spsum = spool_ps.tile([128, 4], F32, tag="spsum")

        score_batch(1, *BATCHES1[0])
        exp_phase(0)
        denom(0)
        shear_mm(0)
        score_batch(1, *BATCHES1[1])
        shear_copy(0, 0)
        shear_copy(0, 1)
        score_batch(1, *BATCHES1[2])
        score_batch(1, *BATCHES1[3])
        edge(1)
        value_mm(0, 0)
        value_out(0, 0, "dve")
        value_mm(0, 1)
        value_out(0, 1, "act")
        exp_phase(1)
        denom(1)
        shear_mm(1)
        shear_copy(1, 0)
        value_mm(1, 0)
        shear_copy(1, 1, "act")
        value_mm(1, 1)
        value_out(1, 0, "dve")
        value_out(1, 1, "dve")
        estack.close()


def build_nc(cdt=_CDT):
    nc = bacc.Bacc("TRN2", target_bir_lowering=False)
    xt = nc.dram_tensor("xt", [128, 4 * HALO], _XDT, kind="ExternalInput")
    xe = nc.dram_tensor("xe", [128, 8 * 513], cdt, kind="ExternalInput")
    wwq = nc.dram_tensor("wwq", [128, 640], cdt, kind="ExternalInput")
    wwr = nc.dram_tensor("wwr", [128, WWR], cdt, kind="ExternalInput")
    mbb = nc.dram_tensor("mbb", [128, 2], F32, kind="ExternalInput")
    vout = nc.dram_tensor("v", [T_LOC, D], _CDT if os.environ.get("ATTN_VOUT", "bf16") == "bf16" else F32, kind="ExternalOutput")
    with tile.TileContext(nc) as tc:
        _emit(nc, tc, cdt, xt, xe, wwq, wwr, mbb, vout)
    nc.compile()
    return nc


# ---------------- host-side prep ----------------

def prep_core_inputs(x, Wt, Wx, bh, Wa, ba, core, cdt=_CDT):
    ndt = _np_dt(cdt)
    xdt = _np_dt(_XDT)
    b, half = core // 2, core % 2
    t0 = half * T_LOC
    lo, hi = t0 - 32, t0 + 544
    pad_lo, pad_hi = max(0, -lo), max(0, hi - T)
    xs = x[b, max(0, lo):min(T, hi), :]
    x_halo = np.pad(xs, ((pad_lo, pad_hi), (0, 0)))     # [576, 512]

    # xt: [128, 4*576], chunk c = x_halo[:, 128c:128c+128].T
    xt = np.empty((128, 4 * HALO), np.float32)
    for c in range(4):
        xt[:, HALO * c:HALO * (c + 1)] = x_halo[:, 128 * c:128 * c + 128].T
    # xe: [128, 8*513], block m = rows [64m, 64m+128) with ones column
    xe_rows = np.concatenate(
        [x_halo, np.ones((HALO, 1), np.float32)], 1)    # [576, 513]
    xe = np.empty((128, 8 * 513), np.float32)
    for m in range(NBLK):
        xe[:, 513 * m:513 * (m + 1)] = xe_rows[64 * m:64 * m + 128, :]
    # wwq: [128, 640] = Wt tiled x4 [512] | Wx [128]
    wwq = np.zeros((128, 640), np.float32)
    for c in range(4):
        wwq[:, 128 * c:128 * c + 128] = np.tile(Wt[128 * c:128 * c + 128, :],
                                                (1, 4))
        wwq[:, 512 + 32 * c:512 + 32 * c + 32] = Wx[128 * c:128 * c + 128, :]
    # wwr: [128, 443] = wa_wide[124] | sh[255] | Rlo[32] | Rhi[32]
    wwr = np.zeros((128, WWR), np.float32)
    for g in range(4):
        wwr[32 * g:32 * g + 32, WA0 + 60 + g] = Wa[:, 0]
    kk = np.arange(128)
    wwr[kk, SH0 + kk + 127] = 1.0
    # edge-mask factors: E[d', t] += -30 where j = t0 + t + d' - 32 invalid.
    # left edge (t0 == 0):  invalid iff t + d' < 32  (d' = k in [0,32))
    # right edge (t0+512 == T): invalid iff t + d' > 543 (d' = k+32)
    ks = np.arange(32)[:, None]
    ts = np.arange(32)[None, :]
    if t0 == 0:
        wwr[0:32, RLO0:RLO0 + 32] = np.where(ts < 32 - ks, -30.0, 0.0)
    if t0 + T_LOC == T:
        wwr[0:32, RHI0:RHI0 + 32] = np.where((480 + ts) + (ks + 32) > 543,
                                             -30.0, 0.0)
    # mbb: [128, 2] = ba (rows 0-63) | bh4
    mbb = np.zeros((128, 2), np.float32)
    mbb[0:64, 0] = float(np.asarray(ba).reshape(-1)[0])
    mbb[:, 1] = np.tile(np.asarray(bh, np.float32), 4)

    return {
        "xt": xt.astype(xdt),
        "xe": xe.astype(ndt),
        "wwq": wwq.astype(ndt),
        "wwr": wwr.astype(ndt),
        "mbb": mbb,
    }


_NC_CACHE = {}


def _get_nc(cdt=_CDT):
    key = str(cdt)
    if key not in _NC_CACHE:
        _NC_CACHE[key] = build_nc(cdt)
    return _NC_CACHE[key]


def kernel(x, Wt, Wx, bh, Wa, ba, _trace=False):
    x = np.asarray(x, np.float32)
    Wt = np.asarray(Wt, np.float32)
    Wx = np.asarray(Wx, np.float32)
    bh = np.asarray(bh, np.float32)
    Wa = np.asarray(Wa, np.float32)
    ba = np.asarray(ba, np.float32)
    nc = _get_nc()
    in_maps = [prep_core_inputs(x, Wt, Wx, bh, Wa, ba, c)
               for c in range(NCORES)]
    res = run_bass_kernel_spmd(nc, in_maps, core_ids=list(range(NCORES)),
                               trace=_trace)
    out = np.empty((B, T, D), np.float32)
    for c in range(NCORES):
        b, half = c // 2, c % 2
        out[b, half * T_LOC:(half + 1) * T_LOC, :] = np.asarray(
            res.results[c]["v"], np.float32)
    if _trace:
        return out, res
    return out
